# revision 1
# baseline (speedup 1.0000x reference)
"""Multi-head self-attention (B=4, S=2048, E=1024, H=16) + residual + layernorm
on 8 Trainium2 NeuronCores — fp8 DoubleRow version.

Sharding: data-parallel over batch (4) x query-split (2-way) = 8 cores (no
collectives; a pairwise-ReduceScatter TP variant measured ~410us for the
collective alone, so K/V are duplicated across the query-split pair).

vs the bf16 baseline (1.5x faster when benched side by side):
- All matmuls run fp8e4 DoubleRow at 0.5 cycles/row. True K>=256
  contractions (Q/K/V projections, PV, WO) pair k-chunks per instruction;
  the K=64 score matmuls use stride-0 "slot" dims (both DoubleRow slots
  read the same q/k data, doubling the product — folded into the exp
  scale) so they also stream 2 columns/cycle.
- Weights pre-scaled on host into fp8 range (x64; x32 for Q/K so 32*q
  stays under e4m3 max 240); descales folded into existing ops (exp scale,
  1/128 reciprocal-broadcast seed, WO epilogue multiplier).
- V bias + WO bias folded into the residual on host:
  x_res' = x + WO_b + WV_b @ WO_w.T (softmax weights sum to 1, so the V
  bias passes through attention exactly).
- Scores land in double-buffered 2-bank PSUM tiles ([128,2,512], one key
  tile each) so the next tile's score matmuls overlap this tile's exp —
  with a single 4-bank quad tile, PE and ACT strictly alternated (~2.5us
  per quad of stall). The kernel is paced by the ScalarE exp throughput
  (~0.87 ns/el measured = 1 el/partition/cycle at 1.2GHz, ~230us/core
  floor for the 262k exps/partition).
- exp writes fp8 directly (feeds fp8 PV); the augmented-V ones column
  (=32*exp(mask)) accumulates the softmax denominator in PSUM row 64.
- rstd via ln+exp (same ACT table as the softmax exp -> no table reloads;
  Sqrt forced ~16 reloads/iter), batched per 4-tile epilogue group.
- LN apply runs on DVE (bf16 SBUF ops hit the 4x DVE mode, ~0.34 ns/el
  measured; the Pool/GPSIMD engine measured 1.85 ns/el and PSUM is
  DVE/ACT-only, so Pool only writes the tiny V ones-columns).
- bf16 residual/LN datapath and bf16 output (converted to f32 on host).
"""
import numpy as np
import ml_dtypes

B, S, E = 4, 2048, 1024
H, D = 16, 64
SQ = S // 2
N_CORES = 8

_CACHE = {}


def _build_nc(unroll=1):
    import concourse.bass as bass
    import concourse.mybir as mybir
    import concourse.tile as tile
    from concourse import bacc

    F32 = mybir.dt.float32
    BF16 = mybir.dt.bfloat16
    FP8 = mybir.dt.float8e4
    AF = mybir.ActivationFunctionType
    DR = mybir.MatmulPerfMode.DoubleRow
    ALU = mybir.AluOpType

    nc = bacc.Bacc("TRN2", target_bir_lowering=False, debug=False,
                   num_devices=N_CORES)

    xT = nc.declare_dram_parameter("xT", [E, S], FP8, isOutput=False)
    xqT = nc.declare_dram_parameter("xqT", [E, SQ], FP8, isOutput=False)
    x_res = nc.declare_dram_parameter("x_res", [SQ, E], BF16, isOutput=False)
    wqT = nc.declare_dram_parameter("wqT", [E, E], FP8, isOutput=False)
    wkT = nc.declare_dram_parameter("wkT", [E, E], FP8, isOutput=False)
    wvT = nc.declare_dram_parameter("wvT", [E, E], FP8, isOutput=False)
    woT = nc.declare_dram_parameter("woT", [E, E], FP8, isOutput=False)
    bq = nc.declare_dram_parameter("bq", [128, 8], F32, isOutput=False)
    bk = nc.declare_dram_parameter("bk", [128, 8], F32, isOutput=False)
    em32 = nc.declare_dram_parameter("em32", [128, 16], F32, isOutput=False)
    ln_w_row = nc.declare_dram_parameter("ln_w_row", [1, E], BF16,
                                         isOutput=False)
    ln_b_row = nc.declare_dram_parameter("ln_b_row", [1, E], BF16,
                                         isOutput=False)

    out_half = nc.declare_dram_parameter("out_half", [SQ, E], BF16,
                                         isOutput=True)

    def bc_ap(param, n):
        return bass.AP(tensor=param, offset=0, ap=[[0, 128], [1, n]])

    with tile.TileContext(nc) as tc:
        with tc.tile_pool(name="persist", bufs=1) as pp, \
             tc.tile_pool(name="psum", bufs=2, space="PSUM") as ps, \
             tc.tile_pool(name="small", bufs=2) as sp:

          for _rep in range(unroll):
            pfx = f"r{_rep}_"

            # ---------- small constants ----------
            bq_t = pp.tile([128, 8], F32, tag="bq")
            nc.sync.dma_start(out=bq_t[:], in_=bq.ap())
            bk_t = pp.tile([128, 8], F32, tag="bk")
            nc.sync.dma_start(out=bk_t[:], in_=bk.ap())
            em_t = pp.tile([128, 16], F32, tag="em")
            nc.sync.dma_start(out=em_t[:], in_=em32.ap())
            # denominator broadcast seed: bcp = den/128 so rec = 128/den,
            # making ctx_t = 128 * ctx_true (fp8 range)
            ones_row = pp.tile([1, 64], BF16, tag="ones_row")
            nc.vector.memset(ones_row[:], 1.0 / 128.0)
            ones16 = pp.tile([128, 16], F32, tag="ones16")
            nc.vector.memset(ones16[:], 1.0)
            eps_t = pp.tile([128, 1], F32, tag="eps")
            nc.vector.memset(eps_t[:], 1e-12)

            # persistent activations
            q_t = pp.tile([128, 8, SQ], FP8, tag="Q")       # 32*q
            k_t = pp.tile([128, 8, S], FP8, tag="K")        # 32*k
            v_t = pp.tile([128, 16, 16, 65], FP8, tag="V")  # 32*em*v
            ctx_t = pp.tile([128, 8, SQ], FP8, tag="ctx")   # 128*ctx

            # ---------- phase 1: Q/K/V projections (fp8 DoubleRow) -------
            with tc.tile_pool(name="w1", bufs=1) as w1:
                xT_t = w1.tile([128, 8, S], FP8, tag="xT")
                xqT_t = w1.tile([128, 8, SQ], FP8, tag="xqT")
                wq_t = w1.tile([128, 8, E], FP8, tag="wq")
                wk_t = w1.tile([128, 8, E], FP8, tag="wk")
                wv_t = w1.tile([128, 8, E], FP8, tag="wv")
                for kt in range(8):
                    nc.sync.dma_start(
                        out=wk_t[:, kt, :],
                        in_=wkT.ap().rearrange("(kt p) m -> p kt m",
                                               p=128)[:, kt, :])
                    nc.sync.dma_start(
                        out=xT_t[:, kt, :],
                        in_=xT.ap().rearrange("(kt p) s -> p kt s",
                                              p=128)[:, kt, :])
                for kt in range(8):
                    nc.sync.dma_start(
                        out=wq_t[:, kt, :],
                        in_=wqT.ap().rearrange("(kt p) m -> p kt m",
                                               p=128)[:, kt, :])
                    nc.sync.dma_start(
                        out=xqT_t[:, kt, :],
                        in_=xqT.ap().rearrange("(kt p) s -> p kt s",
                                               p=128)[:, kt, :])
                    nc.sync.dma_start(
                        out=wv_t[:, kt, :],
                        in_=wvT.ap().rearrange("(kt p) m -> p kt m",
                                               p=128)[:, kt, :])

                # K (full s2 range); PSUM readable only by DVE/ACT
                for mt in range(8):
                    for sb in range(4):
                        p = ps.tile([128, 512], F32, tag="mm")
                        for q in range(4):
                            nc.tensor.matmul(
                                p[:],
                                wk_t[:, 2 * q:2 * q + 2,
                                     mt * 128:(mt + 1) * 128],
                                xT_t[:, 2 * q:2 * q + 2,
                                     sb * 512:(sb + 1) * 512],
                                start=(q == 0), stop=(q == 3), perf_mode=DR)
                        nc.vector.tensor_scalar_add(
                            out=k_t[:, mt, sb * 512:(sb + 1) * 512],
                            in0=p[:], scalar1=bk_t[:, mt:mt + 1])
                # Q: copies+bias on DVE
                for mt in range(8):
                    for sb in range(2):
                        p = ps.tile([128, 512], F32, tag="mm")
                        for q in range(4):
                            nc.tensor.matmul(
                                p[:],
                                wq_t[:, 2 * q:2 * q + 2,
                                     mt * 128:(mt + 1) * 128],
                                xqT_t[:, 2 * q:2 * q + 2,
                                      sb * 512:(sb + 1) * 512],
                                start=(q == 0), stop=(q == 3), perf_mode=DR)
                        nc.vector.tensor_scalar_add(
                            out=q_t[:, mt, sb * 512:(sb + 1) * 512],
                            in0=p[:], scalar1=bq_t[:, mt:mt + 1])

                # V (s2-major, augmented): v_t = (psum*0.5)*em32  (=32*em*v)
                for s2t in range(16):
                    eng = nc.vector
                    for half in range(2):
                        p = ps.tile([128, 512], F32, tag="mm")
                        for q in range(4):
                            nc.tensor.matmul(
                                p[:],
                                xT_t[:, 2 * q:2 * q + 2,
                                     s2t * 128:(s2t + 1) * 128],
                                wv_t[:, 2 * q:2 * q + 2,
                                     half * 512:(half + 1) * 512],
                                start=(q == 0), stop=(q == 3), perf_mode=DR)
                        # psum=64*v; (64v * 1/64) * (32*em) = 32*em*v
                        eng.tensor_scalar(
                            out=v_t[:, s2t, half * 8:(half + 1) * 8, 0:64],
                            in0=p[:].rearrange("p (h d) -> p h d", h=8),
                            scalar1=1.0 / 64.0, scalar2=em_t[:, s2t:s2t + 1],
                            op0=ALU.mult, op1=ALU.mult)
                    # ones column = 32*em (em_t already holds 32*em)
                    nc.gpsimd.tensor_scalar_mul(
                        out=v_t[:, s2t, :, 64],
                        in0=ones16[:, :],
                        scalar1=em_t[:, s2t:s2t + 1])

            # ---------- phase 2+3: attention + fused WO/LN, pipelined ----
            blocks = [(sb1, hm) for sb1 in range(2) for hm in range(8)]
            state = {}

            def emit_scores_half(i, s2t):
                # fp8 DoubleRow with stride-0 slot dims: the PE streams 2
                # moving columns/cycle, both slots read the SAME q/k data,
                # so psum = 2*(32q . 32k) = 2048*q.k at half the cycles.
                # One s2t key tile per st buffer ([128,2,512] = 2 PSUM
                # banks, double-buffered) so the next half's score matmuls
                # overlap this half's exp — the single-buffered [128,4,512]
                # quad tile serialized PE and ACT per quad.
                sb1, hm = blocks[i]
                st = ps.tile([128, 2, 512], F32, tag="st", bufs=2,
                             name=f"st{pfx}{i}_{s2t}")
                s1 = slice(sb1 * 512, (sb1 + 1) * 512)
                for idx, hp in enumerate((0, 64)):
                    lh = k_t[hp:hp + 64, hm,
                             s2t * 128:(s2t + 1) * 128].unsqueeze(
                                 1).broadcast_to((64, 2, 128))
                    rh = q_t[hp:hp + 64, hm, s1].unsqueeze(
                        1).broadcast_to((64, 2, 512))
                    nc.tensor.matmul(
                        st[:, idx, :], lh, rh,
                        start=True, stop=True, perf_mode=DR,
                        tile_position=(hp, 0))
                exp_pair = state[i]["exp"]
                # scores_psum = 2048*(q.k); softmax wants exp(q.k/8)
                nc.scalar.activation(
                    out=exp_pair[:, s2t, :, :], in_=st[:],
                    func=AF.Exp, scale=1.0 / 16384.0)

            def emit_pv_quad(i, q):
                exp_pair = state[i]["exp"]
                pvs = state[i]["pv"]
                for idx in range(2):
                    hl = blocks[i][1] * 2 + idx
                    nc.tensor.matmul(
                        pvs[idx][:],
                        v_t[:, 2 * q:2 * q + 2, hl, :],
                        exp_pair[:, 2 * q:2 * q + 2, idx, :],
                        start=(q == 0), stop=(q == 7), perf_mode=DR)

            def emit_pv_norm(i):
                sb1, hm = blocks[i]
                s1 = slice(sb1 * 512, (sb1 + 1) * 512)
                for idx, hp in enumerate((0, 64)):
                    pv = state[i]["pv"][idx]
                    den = sp.tile([1, 512], BF16, tag="den",
                                  name=f"den{pfx}{i}_{idx}")
                    nc.vector.tensor_copy(out=den[:], in_=pv[64:65, :])
                    bcp = ps.tile([64, 512], F32, tag="mm",
                                  name=f"bcp{pfx}{i}_{idx}")
                    nc.tensor.matmul(bcp[:], ones_row[:], den[:],
                                     start=True, stop=True)
                    rec = sp.tile([64, 512], F32, tag="rec",
                                  name=f"rec{pfx}{i}_{idx}")
                    nc.vector.reciprocal(out=rec[:], in_=bcp[:])
                    nc.vector.tensor_mul(
                        out=ctx_t[hp:hp + 64, hm, s1],
                        in0=pv[0:64, :], in1=rec[:])

            def wo_ln_tile_closures(sb1, wo_t, lnw_bc, lnb_bc, ep):
                # Batched epilogue: 4x stage-A (WO+residual+stats), one
                # ln/exp rstd batch for the group, then 4x stage-B (LN
                # apply + store). Keeps ACT on a single ln+exp table and
                # reduces tiny ACT instruction count.
                gmv = ep.tile([128, 4, 2], F32, tag="gmv",
                              name=f"gmv{pfx}{sb1}")
                rstd_g = ep.tile([128, 4], F32, tag="rstdg",
                                 name=f"rstdg{pfx}{sb1}")
                vs = [ep.tile([128, E], BF16, tag="v", bufs=5,
                              name=f"v{pfx}{sb1 * 4 + ti}")
                      for ti in range(4)]
                cl = [lambda ti=ti: emit_wo_tile_a(
                    sb1 * 4 + ti, ti, wo_t, gmv, vs[ti], ep)
                    for ti in range(4)]
                cl.append(lambda: emit_rstd_group(gmv, rstd_g, sb1))
                cl.extend([lambda ti=ti: emit_ln_tile_b(
                    sb1 * 4 + ti, ti, gmv, rstd_g, vs[ti], lnw_bc, lnb_bc)
                    for ti in range(4)])
                return cl

            def emit_wo_tile_a(st_i, ti, wo_t, gmv, v, ep):
                rows = slice(st_i * 128, (st_i + 1) * 128)
                xr = ep.tile([128, E], BF16, tag="xr", name=f"xr{pfx}{st_i}")
                nc.sync.dma_start(out=xr[:], in_=x_res.ap()[rows, :])
                for eb in range(2):
                    p = ps.tile([128, 512], F32, tag="mm",
                                name=f"wop{pfx}{st_i}_{eb}")
                    for m in range(4):
                        nc.tensor.matmul(
                            p[:],
                            ctx_t[:, 2 * m:2 * m + 2,
                                  st_i * 128:(st_i + 1) * 128],
                            wo_t[:, 2 * m:2 * m + 2,
                                 eb * 512:(eb + 1) * 512],
                            start=(m == 0), stop=(m == 3), perf_mode=DR)
                    # v = psum/8192 + x_res'  (stt illegal on Pool engine)
                    nc.vector.scalar_tensor_tensor(
                        out=v[:, eb * 512:(eb + 1) * 512], in0=p[:],
                        scalar=1.0 / 8192.0,
                        in1=xr[:, eb * 512:(eb + 1) * 512],
                        op0=ALU.mult, op1=ALU.add)
                stats = ep.tile([128, 2, 6], F32, tag="stats",
                                name=f"stats{pfx}{st_i}")
                nc.vector.bn_stats(out=stats[:, 0, :], in_=v[:, 0:512])
                nc.vector.bn_stats(out=stats[:, 1, :], in_=v[:, 512:1024])
                nc.vector.bn_aggr(out=gmv[:, ti, :], in_=stats[:])

            def emit_rstd_group(gmv, rstd_g, sb1):
                # rstd = exp(-0.5*ln(var+eps)); ln+exp share the ACT table
                lnv = sp.tile([128, 4], F32, tag="lnv",
                              name=f"lnv{pfx}{sb1}")
                nc.scalar.activation(out=lnv[:], in_=gmv[:, :, 1],
                                     func=AF.Ln, bias=eps_t[:, 0:1],
                                     scale=1.0)
                nc.scalar.activation(out=rstd_g[:], in_=lnv[:],
                                     func=AF.Exp, scale=-0.5)

            def emit_ln_tile_b(st_i, ti, gmv, rstd_g, v, lnw_bc, lnb_bc):
                # all-bf16 SBUF ops: DVE runs these at 4x rate; the Pool
                # engine measured 5x slower than DVE here.
                rows = slice(st_i * 128, (st_i + 1) * 128)
                u = v  # in-place LN apply
                nc.vector.tensor_scalar(
                    out=u[:], in0=v[:],
                    scalar1=gmv[:, ti, 0:1], scalar2=rstd_g[:, ti:ti + 1],
                    op0=ALU.subtract, op1=ALU.mult)
                nc.vector.tensor_mul(out=u[:], in0=u[:], in1=lnw_bc[:])
                nc.vector.tensor_add(out=u[:], in0=u[:], in1=lnb_bc[:])
                nc.sync.dma_start(out=out_half.ap()[rows, :], in_=u[:])

            with tc.tile_pool(name="attn", bufs=1) as ap_pool, \
                 tc.tile_pool(name="epi", bufs=2) as ep:
                wo_t = ap_pool.tile([128, 8, E], FP8, tag="wo")
                nc.sync.dma_start(out=wo_t[:], in_=woT.ap().rearrange(
                    "(mt p) eo -> p mt eo", p=128))
                lnw_bc = ap_pool.tile([128, E], BF16, tag="lnw_bc")
                nc.sync.dma_start(out=lnw_bc[:], in_=bc_ap(ln_w_row, E))
                lnb_bc = ap_pool.tile([128, E], BF16, tag="lnb_bc")
                nc.sync.dma_start(out=lnb_bc[:], in_=bc_ap(ln_b_row, E))

                wo_queue = []
                for i in range(len(blocks) + 1):
                    if i < len(blocks):
                        state[i] = {
                            "exp": ap_pool.tile([128, 16, 2, 512], FP8,
                                                tag="exp", bufs=2,
                                                name=f"exp{pfx}{i}"),
                            "pv": [ps.tile([65, 512], F32, tag="pv", bufs=2,
                                           name=f"pv{pfx}{i}_{idx}")
                                   for idx in range(2)],
                        }
                    for q in range(8):
                        if i < len(blocks):
                            emit_scores_half(i, 2 * q)
                            emit_scores_half(i, 2 * q + 1)
                        if i > 0:
                            emit_pv_quad(i - 1, q)
                        if wo_queue and q % 2 == 1:
                            wo_queue.pop(0)()
                    if i > 0:
                        emit_pv_norm(i - 1)
                        state.pop(i - 1)
                        if blocks[i - 1][1] == 7:
                            wo_queue.extend(
                                wo_ln_tile_closures(blocks[i - 1][0], wo_t,
                                                    lnw_bc, lnb_bc, ep))
                for fn in wo_queue:
                    fn()

    nc.finalize()
    return nc


def _prepare_in_maps(inputs):
    f8 = ml_dtypes.float8_e4m3
    bf = ml_dtypes.bfloat16
    f32 = np.float32
    x = np.ascontiguousarray(inputs["input_tensor"], dtype=f32)
    mask = np.ascontiguousarray(inputs["mask"], dtype=f32)
    WS = 64.0    # host weight pre-scale into fp8 range (V, O)
    WSQK = 32.0  # Q/K scale: 32*q stays under fp8 e4m3 max (240)
    # V-bias and WO-bias fold into the residual:
    # x + ctx@WO.T + WO_b with ctx = attn + WV_b  ->  x_res' adds
    # WO_b + WV_b @ WO_w.T
    res_bias = (np.asarray(inputs["WO_b"], f32)
                + np.asarray(inputs["WV_b"], f32)
                @ np.asarray(inputs["WO_w"], f32).T).reshape(1, E)
    in_maps = []
    for c in range(N_CORES):
        b, hc = divmod(c, 2)
        m = {
            "xT": np.ascontiguousarray(x[b].T).astype(f8),
            "xqT": np.ascontiguousarray(
                x[b, hc * SQ:(hc + 1) * SQ].T).astype(f8),
            "x_res": (x[b, hc * SQ:(hc + 1) * SQ] + res_bias).astype(bf),
            "wqT": np.ascontiguousarray(
                inputs["WQ_w"].T * WSQK).astype(f8),
            "wkT": np.ascontiguousarray(
                inputs["WK_w"].T * WSQK).astype(f8),
            "wvT": np.ascontiguousarray(
                inputs["WV_w"].T * WS).astype(f8),
            "woT": np.ascontiguousarray(
                inputs["WO_w"].T * WS).astype(f8),
            "bq": np.ascontiguousarray(
                (np.asarray(inputs["WQ_b"], f32) * WSQK).reshape(8, 128).T),
            "bk": np.ascontiguousarray(
                (np.asarray(inputs["WK_b"], f32) * WSQK).reshape(8, 128).T),
            "em32": np.ascontiguousarray(
                32.0 * np.exp(mask[b, 0, 0]).reshape(16, 128).T.astype(f32)),
            "ln_w_row": np.asarray(
                inputs["ln_w"], f32).reshape(1, E).astype(bf),
            "ln_b_row": np.asarray(
                inputs["ln_b"], f32).reshape(1, E).astype(bf),
        }
        in_maps.append({k: np.ascontiguousarray(v) for k, v in m.items()})
    return in_maps


def _run(inputs, trace=False):
    from concourse.bass_utils import run_bass_kernel_spmd

    if "nc" not in _CACHE:
        _CACHE["nc"] = _build_nc()
    in_maps = _prepare_in_maps(inputs)
    res = run_bass_kernel_spmd(_CACHE["nc"], in_maps, list(range(N_CORES)),
                               trace=trace)
    out = np.empty((B, S, E), np.float32)
    for c in range(N_CORES):
        b, hc = divmod(c, 2)
        out[b, hc * SQ:(hc + 1) * SQ] = res.results[c]["out_half"].astype(
            np.float32)
    return out, res


def kernel(**inputs):
    out, _ = _run(inputs, trace=False)
    return out

